# revision 4
# baseline (speedup 1.0000x reference)
"""Bilinear score kernel for TRN2 (8 NeuronCores, data-parallel over batch).

score[b, t, 0] = states[b, t, :] @ W[0] @ context[b, :] + b[0]

Sharding (per spec hint): states/context sharded on B across the 8 cores
(B == 8 -> one batch per core); W and b replicated.

Per-core dataflow:
  1. v = W @ context_b: 8 fused DVE scalar_tensor_tensor ops over natural-
     layout W tiles (i on partitions) -> v as columns vcols[p, c] = v[c*128+p].
  2. vcols -> PE transpose -> vT[8,128] -> 8 tiny SBUF DMAs -> vrow[1,1024]
     -> PE outer-product with a ones column -> vb[128,1024] (v broadcast
     across partitions).
  3. Stream states (16.8 MB) in [128, 4*1024] tiles; one fused DVE
     scalar_tensor_tensor per 1024-chunk computes the dot products:
     accum_out[p] = sum_h states_tile[p, h] * vb[p, h].
  4. Score columns -> PE transpose -> +bias -> single output DMA.

Memory-bound: ~21 MB/core through HBM at ~358 GB/s.
"""

import numpy as np

import concourse.bass as bass
import concourse.tile as tile
from concourse import bacc, mybir
from concourse.bass import ts
from concourse.bass_utils import run_bass_kernel_spmd

B, T, H = 8, 4096, 1024
P = 128          # SBUF partitions
R = 4            # states rows-of-128 per DMA tile -> [128, R*H] = 2 MB tiles
NT = T // (P * R)    # 8 states tiles per core
WR = 4           # W rows-of-128 per DMA tile
WT = H // (P * WR)   # 2 W tiles per core
NCOLS = H // P   # 8 v-columns
NCORES = 8

F32 = mybir.dt.float32

PROFILE = False          # set True (e.g. from test.py) to capture an NTFF trace
LAST_EXEC_NS = None      # filled when PROFILE is True
LAST_RESULTS = None


def _register_ntff_hook():
    """Register the axon NTFF profile hook that the boot shim skips when
    antenv.axon_hooks is absent from the image. Safe no-op on failure."""
    import sys
    import types

    if "antenv.axon_hooks" in sys.modules:
        return True
    try:
        from trn_agent_boot.trn_boot import _ntff_profile_via_ctypes

        hook = _ntff_profile_via_ctypes("/opt/axon/libaxon_pjrt.so")
        if hook is None:
            return False
        mod = types.ModuleType("antenv.axon_hooks")
        mod.get_axon_ntff_profile_hook = lambda: hook
        sys.modules["antenv.axon_hooks"] = mod
        return True
    except Exception:
        return False


def _build_kernel():
    nc = bacc.Bacc(
        "TRN2",
        target_bir_lowering=False,
        debug=False,
        enable_asserts=False,
        num_devices=NCORES,
    )

    states = nc.dram_tensor("states", [T, H], F32, kind="ExternalInput")
    ctxb = nc.dram_tensor("ctxb", [P, H], F32, kind="ExternalInput")
    w = nc.dram_tensor("w", [H, H], F32, kind="ExternalInput")
    biasc = nc.dram_tensor("biasc", [P, 1], F32, kind="ExternalInput")
    ident = nc.dram_tensor("ident", [P, P], F32, kind="ExternalInput")
    out = nc.dram_tensor("scores", [T, 1], F32, kind="ExternalOutput")

    # DRAM views: t = (n*R + r)*P + p  /  i = (d*WR + r)*P + p
    st_ap = states[:, :].rearrange("(n r p) h -> n p r h", r=R, p=P)
    w_ap = w[:, :].rearrange("(d r p) j -> d p r j", r=WR, p=P)
    out_ap = out[:, :].rearrange("(c p) o -> c (p o)", p=P)

    with tile.TileContext(nc) as tc:
        with (
            tc.tile_pool(name="stp", bufs=4) as stp,
            tc.tile_pool(name="wp", bufs=2) as wp,
            tc.tile_pool(name="sm", bufs=1) as sm,
            tc.tile_pool(name="ps", bufs=1, space="PSUM") as ps,
        ):
            ctx_t = sm.tile([P, H], F32)
            nc.sync.dma_start(ctx_t[:, :], ctxb[:, :])
            bias_t = sm.tile([P, 1], F32)
            nc.sync.dma_start(bias_t[:, :], biasc[:, :])
            id_t = sm.tile([P, P], F32)
            nc.sync.dma_start(id_t[:, :], ident[:, :])
            ones_t = sm.tile([1, P], F32)
            nc.vector.memset(ones_t[:, :], 1.0)

            dummy = sm.tile([P, 1], F32)

            # ---- v = W @ context_b (v as columns) ----
            vcols = sm.tile([P, NCOLS], F32)
            for d in range(WT):
                wt = wp.tile([P, WR * H], F32)
                nc.sync.dma_start(
                    wt[:, :].rearrange("p (r j) -> p r j", r=WR), w_ap[d]
                )
                for r in range(WR):
                    c = d * WR + r
                    nc.vector.scalar_tensor_tensor(
                        out=dummy[:, :].broadcast_to((P, H)),
                        in0=wt[:, ts(r, H)],
                        scalar=1.0,
                        in1=ctx_t[:, :],
                        op0=mybir.AluOpType.mult,
                        op1=mybir.AluOpType.mult,
                        accum_out=vcols[:, c : c + 1],
                    )

            # ---- vcols -> vb (v broadcast across partitions) ----
            vT_ps = ps.tile([NCOLS, P], F32)
            nc.tensor.transpose(vT_ps[:, :], vcols[:, :], id_t[:, :])
            vT = sm.tile([NCOLS, P], F32)
            nc.vector.tensor_copy(vT[:, :], vT_ps[:, :])
            vrow = sm.tile([1, H], F32)
            for c in range(NCOLS):
                nc.sync.dma_start(vrow[0:1, ts(c, P)], vT[c : c + 1, :])
            vb_ps = ps.tile([P, H], F32)
            nc.tensor.matmul(
                vb_ps[:, ts(0, 512)], ones_t[0:1, :], vrow[0:1, ts(0, 512)],
                start=True, stop=True,
            )
            nc.tensor.matmul(
                vb_ps[:, ts(1, 512)], ones_t[0:1, :], vrow[0:1, ts(1, 512)],
                start=True, stop=True,
            )
            vb = sm.tile([P, H], F32)
            nc.vector.tensor_copy(vb[:, ts(0, 512)], vb_ps[:, ts(0, 512)])
            nc.vector.tensor_copy(vb[:, ts(1, 512)], vb_ps[:, ts(1, 512)])

            # ---- scores = states_b @ v (columns), then transpose + bias ----
            cols = sm.tile([P, T // P], F32)
            for n in range(NT):
                st = stp.tile([P, R * H], F32)
                nc.sync.dma_start(
                    st[:, :].rearrange("p (r h) -> p r h", r=R), st_ap[n]
                )
                for r in range(R):
                    c = n * R + r
                    nc.vector.scalar_tensor_tensor(
                        out=dummy[:, :].broadcast_to((P, H)),
                        in0=st[:, ts(r, H)],
                        scalar=1.0,
                        in1=vb[:, :],
                        op0=mybir.AluOpType.mult,
                        op1=mybir.AluOpType.mult,
                        accum_out=cols[:, c : c + 1],
                    )

            sT_ps = ps.tile([T // P, P], F32)
            nc.tensor.transpose(sT_ps[:, :], cols[:, :], id_t[:, :])
            sT = sm.tile([T // P, P], F32)
            nc.vector.tensor_scalar_add(sT[:, :], sT_ps[:, :], bias_t[0 : T // P, :])
            nc.sync.dma_start(out_ap, sT[:, :])

    nc.compile()
    return nc


def kernel(states: np.ndarray, context: np.ndarray, W: np.ndarray, b: np.ndarray) -> np.ndarray:
    global LAST_EXEC_NS, LAST_RESULTS

    states = np.asarray(states, dtype=np.float32)
    context = np.asarray(context, dtype=np.float32)
    w2d = np.ascontiguousarray(np.asarray(W, dtype=np.float32)[0])
    bias = np.float32(np.asarray(b, dtype=np.float32)[0])

    ident = np.eye(P, dtype=np.float32)
    bias_col = np.full((P, 1), bias, dtype=np.float32)

    in_maps = []
    for c in range(NCORES):
        in_maps.append(
            {
                "states": np.ascontiguousarray(states[c]),
                "ctxb": np.ascontiguousarray(np.broadcast_to(context[c], (P, H))),
                "w": w2d,
                "biasc": bias_col,
                "ident": ident,
            }
        )

    do_trace = PROFILE and _register_ntff_hook()
    nc = _build_kernel()
    res = run_bass_kernel_spmd(
        nc, in_maps, core_ids=list(range(NCORES)), trace=do_trace
    )
    LAST_EXEC_NS = res.exec_time_ns
    LAST_RESULTS = res

    out = np.stack([res.results[c]["scores"] for c in range(NCORES)], axis=0)
    return out.astype(np.float32)
